# revision 2
# baseline (speedup 1.0000x reference)
"""Sparse KV block gather on 8 Trainium2 NeuronCores.

Problem: kv (32, 2, 64, 49, 256) f32 -> kv_flat (32, 128, 49*256);
out[b, q, k] = kv_flat[b, r_idx[b, q, k]]  -> (32, 64, 8, 49, 256).

Sharding: batch dim n=32 split across 8 cores (4 batches/core).

Strategy (v3, bf16 wire format): the output tolerance (rel 2e-2) admits
bf16 (rounding rel err ~2e-3), so kv is downcast to bf16 on the host and
the whole device pipeline runs bf16, halving HBM traffic per core from
26+103 MB to 13+51 MB (~180 us at the ~358 GB/s HBM-per-NC limit).

Each batch's kv (3.2 MB = 128 blocks x 25 KB) is staged once in SBUF,
one block per partition; all four batches fit simultaneously, so the
loads are queued at t=0 with no compute-dependent waits.  The gather is
a dynamic partition permutation on TensorE as bf16 matmuls against
one-hot selection matrices (exact: 1.0*x accumulated in fp32 PSUM).
PSUM tiles are drained (and downcast to bf16) to SBUF by VectorE and
ScalarE alternately, then written to HBM by HWDGE DMA.  The host
upcasts the bf16 result to f32.

The one-hot matrices are a pure host-side re-encoding of r_idx (0.5 MB
per core); kv itself is shipped raw (as bf16).
"""

import ml_dtypes
import numpy as np

import concourse.bacc as bacc
import concourse.bass as bass
import concourse.mybir as mybir
from concourse._compat import get_trn_type
from concourse.bass_utils import run_bass_kernel_spmd

# Problem shapes (hardcoded per contract: kernel.py is self-contained).
N, V, P2, W2, CKV = 32, 2, 64, 49, 256
TOPK = 8
NCORES = 8
NB = N // NCORES             # 4 batches per core
BLOCKS = V * P2              # 128 source blocks per batch
ELEM = W2 * CKV              # 12544 elems per block (25088 B bf16)
IDX_PER_B = P2 * TOPK        # 512 gathered blocks per batch
JCHUNK = 128                 # output blocks per one-hot matmul group
NJC = IDX_PER_B // JCHUNK    # 4 j-chunks per batch
FT = 448                     # f-columns per matmul tile (12544 = 28*448)
NFT = ELEM // FT             # 28 tiles per j-chunk
HALF = NFT // 2              # 14 tiles per DMA-out half (6272 elems)
NT = NB * NJC * NFT          # 448 matmul tiles per core
NG = NT // HALF              # 32 DMA-out groups per core

BF16 = mybir.dt.bfloat16

_CACHE = {}


def _build_nc():
    nc = bacc.Bacc(get_trn_type() or "TRN2")
    kv_in = nc.dram_tensor(
        "kv", [NB, BLOCKS, ELEM], BF16, kind="ExternalInput"
    )
    oh_in = nc.dram_tensor(
        "oh", [128, NB * NJC * JCHUNK], BF16, kind="ExternalInput"
    )
    out = nc.dram_tensor(
        "out", [NB, NJC, JCHUNK, ELEM], BF16, kind="ExternalOutput"
    )

    with (
        nc.sbuf_tensor("kv_sb", [128, NB, ELEM], BF16) as kv_sb,
        nc.sbuf_tensor("oh_sb", [128, NB * NJC * JCHUNK], BF16) as oh_sb,
        nc.sbuf_tensor("stage", [128, 2, HALF * FT], BF16) as stage,
        nc.psum_tensor("ps", [128, 8, 512], mybir.dt.float32) as ps,
        nc.semaphore("s_oh") as s_oh,
        nc.semaphore("s_ld") as s_ld,
        nc.semaphore("s_mm") as s_mm,
        nc.semaphore("s_drv") as s_drv,   # DVE drains (even tiles)
        nc.semaphore("s_dra") as s_dra,   # ACT drains (odd tiles)
        nc.semaphore("s_out") as s_out,
        nc.Block() as block,
    ):

        # kv load segments (k-tile ranges) per batch; batch 0 leads with a
        # small sliver so the first matmul can start almost immediately.
        # All batches have their own SBUF buffer, so every load is queued
        # up-front with no waits.
        segs = []  # (n, k0, k1)
        for n in range(NB):
            bounds = [0, 2, 7, 14, 21, 28] if n == 0 else [0, 14, 28]
            for k0, k1 in zip(bounds, bounds[1:]):
                segs.append((n, k0, k1))
        seg_of = {}  # (n, k0) -> 1-based seg count when loaded
        for i, (n, k0, k1) in enumerate(segs):
            seg_of[(n, k0)] = i + 1

        @block.gpsimd
        def _(gpsimd):
            for n, k0, k1 in segs:
                gpsimd.dma_start(
                    out=kv_sb[:, n, k0 * FT : k1 * FT],
                    in_=kv_in[n][:, k0 * FT : k1 * FT],
                ).then_inc(s_ld, 16)

        @block.tensor
        def _(tensor):
            tensor.wait_ge(s_oh, 16)
            for t in range(NT):
                n = t // (NJC * NFT)
                c = (t // NFT) % NJC
                k = t % NFT
                if t == NFT:
                    # one-hots beyond the first j-chunk arrive in load 2
                    tensor.wait_ge(s_oh, 32)
                if c == 0 and (n, k) in seg_of:
                    tensor.wait_ge(s_ld, 16 * seg_of[(n, k)])
                if t >= 8:
                    # PSUM bank t%8 free once drain t-8 completed
                    td = t - 8
                    if td % 2 == 0:
                        tensor.wait_ge(s_drv, td // 2 + 1)
                    else:
                        tensor.wait_ge(s_dra, td // 2 + 1)
                tensor.matmul(
                    ps[:, t % 8, 0:FT],
                    oh_sb[:, (n * NJC + c) * JCHUNK : (n * NJC + c + 1) * JCHUNK],
                    kv_sb[:, n, k * FT : (k + 1) * FT],
                    start=True,
                    stop=True,
                ).then_inc(s_mm, 1)

        def _drain(eng, parity, sem):
            for t in range(parity, NT, 2):
                g = t // HALF
                kk = t % HALF
                eng.wait_ge(s_mm, t + 1)
                if g >= 2:
                    # stage slot g%2 free once DMA-out g-2 done
                    eng.wait_ge(s_out, 16 * (g - 1))
                eng_copy = (
                    eng.tensor_copy if parity == 0 else eng.copy
                )
                eng_copy(
                    stage[:, g % 2, kk * FT : (kk + 1) * FT],
                    ps[:, t % 8, 0:FT],
                ).then_inc(sem, 1)

        @block.vector
        def _(vector):
            _drain(vector, 0, s_drv)

        @block.scalar
        def _(scalar):
            _drain(scalar, 1, s_dra)

        @block.sync
        def _(sync):
            # first j-chunk's one-hot first (32 KB) so matmuls start early
            sync.dma_start(
                out=oh_sb[:, 0:JCHUNK], in_=oh_in[:, 0:JCHUNK]
            ).then_inc(s_oh, 16)
            sync.dma_start(
                out=oh_sb[:, JCHUNK:], in_=oh_in[:, JCHUNK:]
            ).then_inc(s_oh, 16)
            n_outs = 0
            for g in range(NG):
                t0 = g * HALF
                n = t0 // (NJC * NFT)
                c = (t0 // NFT) % NJC
                h = (t0 % NFT) // HALF
                f0 = h * HALF * FT
                # final group: two smaller DMAs to shorten the tail
                pieces = [(0, HALF)] if g < NG - 1 else [(0, HALF // 2), (HALF // 2, HALF)]
                for p0, p1 in pieces:
                    # drains of tiles t0..t0+p1-1 must have completed
                    sync.wait_ge(s_drv, (t0 + p1 + 1) // 2)
                    sync.wait_ge(s_dra, (t0 + p1) // 2)
                    sync.dma_start(
                        out=out[n, c, :, f0 + p0 * FT : f0 + p1 * FT],
                        in_=stage[:, g % 2, p0 * FT : p1 * FT],
                    ).then_inc(s_out, 16)
                    n_outs += 1
            sync.wait_ge(s_out, 16 * n_outs)

    nc.compile()
    return nc


def _prep_onehot(r_idx_core: np.ndarray) -> np.ndarray:
    """r_idx_core: (NB, P2, TOPK) -> one-hot lhsT in SBUF layout
    (128, NB*NJC*JCHUNK) bf16:  arr[i, g*128 + j] = 1 iff r_idx_flat[g, j] == i.
    """
    idx = r_idx_core.reshape(NB * NJC, JCHUNK).astype(np.int64)
    oh = np.zeros((NB * NJC, 128, JCHUNK), np.float32)
    g = np.arange(NB * NJC)[:, None]
    j = np.arange(JCHUNK)[None, :]
    oh[g, idx, j] = 1.0
    return np.ascontiguousarray(
        oh.transpose(1, 0, 2).reshape(128, NB * NJC * JCHUNK)
    ).astype(ml_dtypes.bfloat16)


def make_in_maps(r_idx: np.ndarray, kv: np.ndarray) -> list:
    kv_r = np.asarray(kv, dtype=np.float32).reshape(N, BLOCKS, ELEM)
    kv_bf = kv_r.astype(ml_dtypes.bfloat16)
    in_maps = []
    for c in range(NCORES):
        lo = c * NB
        in_maps.append(
            {
                "kv": np.ascontiguousarray(kv_bf[lo : lo + NB]),
                "oh": _prep_onehot(np.asarray(r_idx)[lo : lo + NB]),
            }
        )
    return in_maps


def kernel(r_idx: np.ndarray, r_weight: np.ndarray, kv: np.ndarray) -> np.ndarray:
    if "nc" not in _CACHE:
        _CACHE["nc"] = _build_nc()
    nc = _CACHE["nc"]

    in_maps = make_in_maps(r_idx, kv)
    res = run_bass_kernel_spmd(nc, in_maps, core_ids=list(range(NCORES)))
    outs = [
        np.asarray(res.results[c]["out"])
        .astype(np.float32)
        .reshape(NB, P2, TOPK, W2, CKV)
        for c in range(NCORES)
    ]
    return np.concatenate(outs, axis=0)
